# revision 5
# baseline (speedup 1.0000x reference)
"""LSTM (PostureRNN) Trainium2 kernel.

Model: 1-layer LSTM (input=1, hidden=64) over [B=1024, T=2048, 1], take the
last hidden state, apply a 64->3 FC.  Data-parallel over 8 NeuronCores
(128 batch rows per core); LSTM/FC weights replicated.

Math reformulation used on-device (validated exactly against the reference):
  - gates computed batch-major [128, 256] by a single K=65 matmul
    (lhsT rows = [2*h^T ; ones], rhs = fused weight matrix R) plus a K=1
    accumulating matmul that injects the x_t contribution (outer product).
  - all activations via a single Tanh call: sigmoid(z) = (1+tanh(z/2))/2 is
    folded into the weights (i,f,o columns pre-scaled by 0.5) and into
    fused scalar_tensor_tensor ops:
        u  = (f'+1) * C2          (C2 = 2c)
        v  = (i'+1) * g
        C2'= 0.5*u + v            (= 2c')
        T  = tanh(0.5 * C2')      (= tanh(c'))
        h~ = (o'+1) * T           (= 2h; the 2x is folded into R / Rfc)
  - h~ is transposed back to feature-major each step with a PE transpose.
"""

import sys

sys.path.insert(0, "/opt/trn_rl_repo")

import numpy as np

import concourse.bacc as bacc
import concourse.tile as tile
from concourse import mybir
from concourse.bass_utils import run_bass_kernel_spmd

F32 = mybir.dt.float32

NCORES = 8
B = 1024
T = 2048
H = 64
O = 3
BL = B // NCORES  # 128 batch rows per core
G4 = 4 * H  # 256


def build_nc(seq_len=T):
    assert seq_len % 128 == 0

    nc = bacc.Bacc("TRN2", target_bir_lowering=False, debug=False)

    xs_d = nc.dram_tensor("xs", [seq_len // 64, 64 * BL], F32, kind="ExternalInput")
    r_d = nc.dram_tensor("r", [H + 2, G4], F32, kind="ExternalInput")
    rfc_d = nc.dram_tensor("rfc", [H + 2, O], F32, kind="ExternalInput")
    eye_d = nc.dram_tensor("eye", [128, 128], F32, kind="ExternalInput")
    out_d = nc.dram_tensor("out", [BL, O], F32, kind="ExternalOutput")

    with tile.TileContext(nc) as tc:
        with (
            tc.tile_pool(name="singles", bufs=1) as singles,
            tc.tile_pool(name="gates_sb", bufs=3) as gates_sb,
            tc.tile_pool(name="work", bufs=3) as work,
            tc.tile_pool(name="xring", bufs=2) as xring,
            tc.tile_pool(name="gpsum", bufs=2, space="PSUM") as gpsum,
            tc.tile_pool(name="tpsum", bufs=2, space="PSUM") as tpsum,
        ):
            r = singles.tile([H + 2, G4], F32, tag="r")
            rfc = singles.tile([H + 2, O], F32, tag="rfc")
            eye = singles.tile([128, 128], F32, tag="eye")
            out_sb = singles.tile([BL, O], F32, tag="out_sb")

            nc.gpsimd.dma_start(r[:], r_d[:])
            nc.gpsimd.dma_start(rfc[:], rfc_d[:])
            nc.gpsimd.dma_start(eye[:], eye_d[:])

            # Persistent state, manually double-buffered.
            hT = [singles.tile([H + 2, BL], F32, tag=f"hT{k}", name=f"hT{k}") for k in range(2)]
            c2 = [singles.tile([BL, H], F32, tag=f"c2{k}", name=f"c2{k}") for k in range(2)]
            for k in range(2):
                nc.vector.memset(hT[k][0:H, :], 0.0)
                # row 64 = x_t slot (rewritten each step), row 65 = ones
                nc.vector.memset(hT[k][H : H + 2, :], 1.0)
                nc.vector.memset(c2[k][:], 0.0)

            for blk in range(seq_len // 64):
              xr = xring.tile([1, 64 * BL], F32, tag="xr")
              nc.sync.dma_start(xr[:], xs_d[blk : blk + 1, :])
              for j in range(64):
                t = blk * 64 + j
                cur, nxt = t % 2, (t + 1) % 2

                gp = gpsum.tile([BL, G4], F32, tag="gp")
                # stage x_t (from the partition-0 ring) into lhsT row 64
                nc.gpsimd.tensor_copy(
                    hT[cur][H : H + 1, :],
                    xr[0:1, BL * j : BL * (j + 1)],
                )
                # gates = [2h^T ; x_t ; ones]^T @ R   (K=66)
                nc.tensor.matmul(gp[:], hT[cur][:], r[:], start=True, stop=True)

                s = gates_sb.tile([BL, G4], F32, tag="s")
                nc.scalar.activation(s[:], gp[:], mybir.ActivationFunctionType.Tanh)

                u = work.tile([BL, H], F32, tag="u")
                v = work.tile([BL, H], F32, tag="v")
                tt = work.tile([BL, H], F32, tag="tt")
                ht = work.tile([BL, H], F32, tag="ht")

                # u = (f' + 1) * C2
                nc.vector.scalar_tensor_tensor(
                    u[:], s[:, H : 2 * H], 1.0, c2[cur][:],
                    op0=mybir.AluOpType.add, op1=mybir.AluOpType.mult,
                )
                # v = (i' + 1) * g
                nc.vector.scalar_tensor_tensor(
                    v[:], s[:, 0:H], 1.0, s[:, 3 * H : 4 * H],
                    op0=mybir.AluOpType.add, op1=mybir.AluOpType.mult,
                )
                # C2' = 0.5*u + v
                nc.vector.scalar_tensor_tensor(
                    c2[nxt][:], u[:], 0.5, v[:],
                    op0=mybir.AluOpType.mult, op1=mybir.AluOpType.add,
                )
                # T = tanh(c') = tanh(0.5 * C2')
                nc.scalar.activation(
                    tt[:], c2[nxt][:], mybir.ActivationFunctionType.Tanh, scale=0.5
                )
                # h~ = (o' + 1) * T
                nc.vector.scalar_tensor_tensor(
                    ht[:], s[:, 2 * H : 3 * H], 1.0, tt[:],
                    op0=mybir.AluOpType.add, op1=mybir.AluOpType.mult,
                )
                # transpose h~ back to feature-major for the next step's lhsT
                tp = tpsum.tile([H, BL], F32, tag="tp")
                nc.tensor.transpose(tp[:], ht[:], eye[:])
                nc.vector.tensor_copy(hT[nxt][0:H, :], tp[:])

            fcp = tpsum.tile([BL, O], F32, tag="fcp")
            last = seq_len % 2
            nc.tensor.matmul(fcp[:], hT[last][:], rfc[:], start=True, stop=True)
            nc.scalar.copy(out_sb[:], fcp[:])
            nc.gpsimd.dma_start(out_d[:], out_sb[:])

    nc.compile()
    return nc


def prep_inputs(x, W_ih, W_hh, b_ih, b_hh, W_fc, b_fc, seq_len=T):
    """Host-side prep: shard x, build fused weight matrices (replicated)."""
    x = np.asarray(x, np.float32).reshape(B, seq_len)
    W_ih = np.asarray(W_ih, np.float32)
    W_hh = np.asarray(W_hh, np.float32)
    b_ih = np.asarray(b_ih, np.float32)
    b_hh = np.asarray(b_hh, np.float32)
    W_fc = np.asarray(W_fc, np.float32)
    b_fc = np.asarray(b_fc, np.float32)

    # gate reorder: torch [i,f,g,o] -> [i,f,o,g]
    perm = np.concatenate(
        [np.arange(0, H), np.arange(H, 2 * H), np.arange(3 * H, 4 * H),
         np.arange(2 * H, 3 * H)]
    )
    cs = np.array([0.5] * (3 * H) + [1.0] * H, np.float32)

    r = np.zeros((H + 2, G4), np.float32)
    r[0:H, :] = W_hh[perm, :].T * cs[None, :] * 0.5
    r[H, :] = W_ih[perm, 0] * cs
    r[H + 1, :] = (b_ih + b_hh)[perm] * cs
    rfc = np.zeros((H + 2, O), np.float32)
    rfc[0:H, :] = W_fc.T * 0.5
    rfc[H + 1, :] = b_fc
    eye = np.eye(128, dtype=np.float32)

    in_maps = []
    for core in range(NCORES):
        xc = x[core * BL : (core + 1) * BL, :]  # [128, T]
        # xs[blk, j*128 + b] = xc[b, blk*64 + j]
        xstage = np.ascontiguousarray(xc.T.reshape(seq_len // 64, 64 * BL))
        in_maps.append({"xs": xstage, "r": r, "rfc": rfc, "eye": eye})
    return in_maps


_NC_CACHE = {}


def run(inputs, seq_len=T, trace=False):
    if seq_len not in _NC_CACHE:
        _NC_CACHE[seq_len] = build_nc(seq_len)
    nc = _NC_CACHE[seq_len]
    in_maps = prep_inputs(
        inputs["x"], inputs["W_ih"], inputs["W_hh"], inputs["b_ih"],
        inputs["b_hh"], inputs["W_fc"], inputs["b_fc"], seq_len=seq_len,
    )
    res = run_bass_kernel_spmd(nc, in_maps, list(range(NCORES)), trace=trace)
    out = np.concatenate([res.results[i]["out"] for i in range(NCORES)], axis=0)
    return out.astype(np.float32), res


def kernel(**inputs):
    out, _ = run(inputs)
    return out
